# revision 19
# baseline (speedup 1.0000x reference)
"""Trainium2 Bass kernel for a single-head causal attention block.

Reference computation (B=4, T=2048, C=1024, H=64):
    q = x @ Wq; k = x @ Wk; v = x @ Wv          # [B,T,H]
    scores = (q @ k^T) * C**-0.5                # causal masked
    out = softmax(scores) @ v                   # [B,T,H]

Sharding: 2 cores per batch (8 cores, B=4). Core (b, t) owns the 4
interleaved 256-row query chunks {t, t+2, t+4, t+6} of batch b, which
balances causal work exactly across the pair. One uniform SPMD program;
all per-core differences are input data (row arrangement + 0/1 masks).

On-chip pipeline (bf16 on-chip, fp32 PSUM accumulate; rel err ~5e-3):
  x is pre-transposed on the host, so x^T streams in as plain contiguous
  DMAs, split across the SP and ACT HWDGE queues -> K^T/V^T + Q^T
  projections off one packed [Wq|Wk|Wv] stationary -> per-512-row-half
  exchange of the packed [K^T|V^T] with the pair partner: AllGather
  through a flat DRAM buffer (fake_cc: equivalent-volume local DMAs for
  the benchmark loop; half 0 runs on the ACT queue, half 1 on SP) ->
  rank-ordered [even-chunk | odd-chunk] kv layout, uniform across
  cores -> V^T unpacked to V-natural via PE transposes -> S^T = K.Q^T
  tiles [s=128, q=256] -> one exp per pair group on ScalarE (1/32 scale
  folded in; logits tiny, no max-subtraction) -> causal masks via DVE
  multiplies -> P^T.[V|1|0] matmul accumulates out^T and the softmax
  denominator together -> PE transpose + reciprocal -> bf16 y rows
  staged and stored via the idle Pool/SWDGE queue.
"""

import numpy as np
import ml_dtypes

B, T, C, H = 4, 2048, 1024, 64
NCORES = 8
P = 128          # partitions
NCB = C // P     # 8 channel blocks
QB = 256         # query block width
TQ = T // 2      # own query rows per core
SCALE = float(C) ** -0.5
BF16 = ml_dtypes.bfloat16

# flat exchange payload per 512-row half: [K^T | V^T] = 128x512 bf16
EX_ELE = 128 * 512

_CACHE = {}


def _build_program(loop_n=1, fake_cc=False, stage="full"):
    # fake_cc: replace the AllGather with equivalent-volume local DMAs —
    # numerically wrong (peer half duplicated) but timing-equivalent; used
    # only by the For_i benchmark loop, where real collectives desync.
    import contextlib
    import concourse.bacc as bacc
    import concourse.mybir as mybir
    from concourse import tile

    f32 = mybir.dt.float32
    bf16 = mybir.dt.bfloat16
    EXP = mybir.ActivationFunctionType.Exp

    nc = bacc.Bacc("TRN2", target_bir_lowering=False, debug=False,
                   num_devices=NCORES)

    xqT_d = nc.dram_tensor("xqT", [C, TQ], bf16, kind="ExternalInput").ap()
    w_d = nc.dram_tensor("wqkv", [P, NCB * 3 * H], bf16,
                         kind="ExternalInput").ap()
    # consts: [iden 128 | masks 4x256] packed in one tensor/DMA
    const_d = nc.dram_tensor("consts", [P, P + 4 * QB], bf16,
                             kind="ExternalInput").ap()
    y_d = nc.dram_tensor("y", [TQ, H], bf16, kind="ExternalOutput").ap()

    with tile.TileContext(nc) as tc:
        with (
            tc.tile_pool(name="const", bufs=1) as constp,
            tc.tile_pool(name="big", bufs=1) as bigp,
            tc.tile_pool(name="exps", bufs=3) as expp,
            tc.tile_pool(name="small", bufs=4) as smallp,
            tc.tile_pool(name="pt", bufs=2, space="PSUM") as psum_t,
            tc.tile_pool(name="psc", bufs=2, space="PSUM") as psum_sc,
            tc.tile_pool(name="po", bufs=2, space="PSUM") as psum_o,
            tc.tile_pool(name="dram", bufs=1, space="DRAM") as dramp,
        ):
          with (tc.For_i(0, loop_n, 1,
                         hint_engines=(mybir.EngineType.PE,
                                       mybir.EngineType.SP,
                                       mybir.EngineType.Activation,
                                       mybir.EngineType.DVE,
                                       mybir.EngineType.Pool))
                if loop_n > 1 else contextlib.nullcontext()):
            w_s = constp.tile([P, NCB, 3 * H], bf16)
            nc.gpsimd.dma_start(
                w_s[:], w_d.rearrange("p (c o) -> p c o", c=NCB))
            consts = constp.tile([P, P + 4 * QB], bf16, name="consts_s",
                                 tag="consts_s")
            nc.gpsimd.dma_start(consts[:], const_d)
            iden = consts[:, 0:P]
            mask_s = {m: consts[:, P + j * QB:P + (j + 1) * QB]
                      for j, m in enumerate("abcd")}
            zbias = constp.tile([P, 1], f32)
            nc.vector.memset(zbias[:], 0.0)
            # warm the ACT exp table-set early (one-time ~2.7us DMA load
            # otherwise lands on the attention critical path)
            expwarm = constp.tile([P, 1], f32)
            nc.scalar.activation(expwarm[:], zbias[:], EXP, bias=zbias[:])

            # ---- x^T: plain loads of the host-transposed x ----
            # [c_part, cb, own_row]; each (cb, half) is one [128, 512]
            # contiguous-row DMA; cb 0-3 ride the SP queue, 4-7 the ACT
            # queue, so the two HWDGE rings fill in parallel.
            xT = bigp.tile([P, NCB, TQ], bf16, name="xT", tag="xT")

            def load_x_half(h):
                for cb in range(NCB):
                    eng = nc.sync if cb < 4 else nc.scalar
                    eng.dma_start(
                        xT[:, cb, h * 512:(h + 1) * 512],
                        xqT_d[cb * P:(cb + 1) * P, h * 512:(h + 1) * 512])

            if stage != "attn":
                load_x_half(0)
                load_x_half(1)

            # ---- projections + per-half [K^T|V^T] exchange ----
            kvT = bigp.tile([P, 2, TQ], bf16, name="kvT", tag="kvT")
            qT = bigp.tile([H, TQ], bf16, name="qT", tag="qT")
            kv_own = bigp.tile([P, TQ], bf16, name="kv_own", tag="kv_own")
            vp = {par: bigp.tile([P, NCB, H + 2], bf16, name=f"vp{par}",
                                 tag=f"vp{par}") for par in range(2)}
            for par in range(2):
                nc.vector.memset(vp[par][:, :, H:H + 1], 1.0)
                nc.vector.memset(vp[par][:, :, H + 1:H + 2], 0.0)

            in_cc = [dramp.tile([1, EX_ELE], bf16, name=f"incc{h}",
                                tag=f"incc{h}") for h in range(2)]
            out_cc = [dramp.tile([2, EX_ELE], bf16, name=f"outcc{h}",
                                 tag=f"outcc{h}") for h in range(2)]

            if stage == "attn":
                nc.vector.memset(kvT[:, :, :], 0.01)
                nc.vector.memset(qT[:, :], 0.01)
                nc.vector.memset(kv_own[:, :], 0.01)
            for h in range(([] if stage == "attn" else [0, 1]) and 2 or 0) if False else range(0 if stage == "attn" else 2):
                cols = slice(h * 512, (h + 1) * 512)
                # exchange half 0 on the ACT ring (idle pre-attention),
                # half 1 on SP (free once x is in)
                ex_eng = nc.scalar if h == 0 else nc.sync
                # K^T/V^T packed projection: psum [k|v, 512] f32
                pkv = psum_t.tile([P, 512], f32, tag="pt")
                for cb in range(NCB):
                    nc.tensor.matmul(
                        pkv[:], w_s[:, cb, H:3 * H], xT[:, cb, cols],
                        start=(cb == 0), stop=(cb == NCB - 1))
                nc.vector.tensor_copy(kv_own[:, cols], pkv[:])
                # Q^T projection
                pq = psum_t.tile([H, 512], f32, tag="pt", name=f"pq{h}")
                for cb in range(NCB):
                    nc.tensor.matmul(
                        pq[:], w_s[:, cb, 0:H], xT[:, cb, cols],
                        start=(cb == 0), stop=(cb == NCB - 1))
                nc.scalar.copy(qT[:, cols], pq[:])

                # ship this half's [K^T | V^T] to the pair partner
                in_v = in_cc[h][0:1, :].rearrange("a (p q) -> (a p) q", p=P)
                if fake_cc:
                    # equivalent-volume local writes standing in for the
                    # AllGather's two output-shard writes
                    for par in range(2):
                        ex_eng.dma_start(
                            out_cc[h][par:par + 1, :].rearrange(
                                "a (p q) -> (a p) q", p=P),
                            kv_own[:, cols])
                    ex_eng.dma_start(in_v, kv_own[:, cols])
                else:
                    ex_eng.dma_start(in_v, kv_own[:, cols])
                    nc.gpsimd.collective_compute(
                        "AllGather",
                        mybir.AluOpType.bypass,
                        replica_groups=[[2 * b, 2 * b + 1]
                                        for b in range(NCORES // 2)],
                        ins=[in_cc[h].opt()],
                        outs=[out_cc[h].opt()],
                    )
                # rank-ordered kv: [:, 0, :] even-chunk [K^T|V^T], [:, 1, :]
                # odd — both shards land in one DMA
                ex_eng.dma_start(
                    kvT[:, 0:2, cols],
                    out_cc[h][0:2, :].rearrange("r (p q) -> p r q", p=P))

            # V^T -> V natural into vp (8 PE transposes of [64,128] per
            # half). h=0 is emitted here; h=1 is deferred into the pair
            # stream (after pair (2,0)'s scores) so its exchange-load
            # dependency does not stall earlier attention work in the
            # in-order PE queue.
            def emit_vt(h):
                for par in range(2):
                    for u in range(4):
                        pv = psum_t.tile([P, H], bf16, tag="pt",
                                         name=f"pv{h}{par}{u}")
                        nc.tensor.transpose(
                            pv[:], kvT[H:2 * H, par, h * 512 + u * P:
                                       h * 512 + (u + 1) * P],
                            iden[H:2 * H, H:2 * H])
                        nc.vector.tensor_copy(
                            vp[par][:, 4 * h + u, 0:H], pv[:])

            emit_vt(0)

            if stage == "front":
                ysf = bigp.tile([P, 16], bf16, name="ysf", tag="ysf")
                nc.vector.tensor_copy(ysf[:], kvT[:, 0, 0:16])
                nc.sync.dma_start(y_d[0:P, 0:16], ysf[:])
            # ---- attention: software-pipelined across (qb, group) ----
            # The PE runs in program order, so scores for stage p+1 are
            # emitted BEFORE the PV matmuls of stage p; the exp/mask chain
            # of stage p then overlaps the next scores group instead of
            # stalling the PE.
            pairs = []
            for i in range(4):
                # unit list: even-chunk units 0..2i+1 then odd-chunk units
                seq = ([(0, u) for u in range(2 * i + 2)]
                       + [(1, u) for u in range(2 * i + 2)])
                for g in range(i + 1):
                    pairs.append((i, g, i + 1, seq))

            po_t = {}
            es_t = {}

            def emit_scores(p):
                i, g, G, seq = pairs[p]
                ps = psum_sc.tile([P, 4 * QB], f32, tag="ps", name=f"ps{p}")
                for k in range(4):
                    par, u = seq[4 * g + k]
                    nc.tensor.matmul(
                        ps[:, k * QB:(k + 1) * QB],
                        kvT[0:H, par, u * P:(u + 1) * P],
                        qT[0:H, i * QB:(i + 1) * QB],
                        start=True, stop=True)
                es = expp.tile([P, 4 * QB], bf16, tag="es", name=f"es{p}")
                if stage == "noexp":
                    nc.vector.tensor_copy(es[:], ps[:])
                elif stage == "smallexp":
                    for eh in range(2):
                        sl = slice(eh * 2 * QB, (eh + 1) * 2 * QB)
                        nc.scalar.activation(es[:, sl], ps[:, sl], EXP,
                                             bias=zbias[:], scale=SCALE)
                else:
                    nc.scalar.activation(es[:], ps[:], EXP,
                                         bias=zbias[:], scale=SCALE)
                if stage != "nomask":
                    for k in range(4):
                        pos = 4 * g + k
                        m = {2 * i: "a", 2 * i + 1: "b",
                             4 * i + 2: "c", 4 * i + 3: "d"}.get(pos)
                        if m is not None:
                            sl = es[:, k * QB:(k + 1) * QB]
                            nc.vector.tensor_mul(sl, sl, mask_s[m])
                es_t[p] = es

            def emit_pv(p):
                i, g, G, seq = pairs[p]
                if g == 0:
                    po_t[i] = psum_o.tile([H + 2, QB], f32, tag="po",
                                          name=f"po{i}")
                es = es_t.pop(p)
                for k in range(4):
                    par, u = seq[4 * g + k]
                    nc.tensor.matmul(
                        po_t[i][:], vp[par][:, u, 0:H + 2],
                        es[:, k * QB:(k + 1) * QB],
                        start=(g == 0 and k == 0),
                        stop=(g == G - 1 and k == 3))
                if g == G - 1:
                    # release po promptly (DVE copy), but DEFER the PE
                    # transpose + output math so it does not block the
                    # next scores group in the PE's in-order stream
                    po = po_t.pop(i)
                    ot = smallp.tile([H + 2, QB], bf16, tag="ot",
                                     name=f"ot{i}")
                    nc.vector.tensor_copy(ot[:], po[:])
                    ot_t[i] = ot

            ot_t = {}
            ys = bigp.tile([P, NCB, H], bf16, name="ys", tag="ys")

            def emit_out(i, ot):
                for h2 in range(2):
                    pt2 = psum_t.tile([P, H + 2], bf16, tag="pt",
                                      name=f"pt2_{i}_{h2}")
                    nc.tensor.transpose(
                        pt2[:], ot[0:H + 2, h2 * P:(h2 + 1) * P],
                        iden[0:H + 2, 0:H + 2])
                    rc = smallp.tile([P, 1], f32, tag="rc",
                                     name=f"rc{i}{h2}")
                    nc.vector.reciprocal(rc[:], pt2[:, H:H + 1])
                    nc.vector.tensor_scalar_mul(
                        ys[:, 2 * i + h2, :], pt2[:, 0:H], rc[:])

            yv = y_d.rearrange("(c p) h -> p c h", p=P)
            if stage == "front":
                pairs = []
            # depth-2 software pipeline: the PE stream runs scores two
            # pairs ahead of the PV consumers, so exp(p)+masks(p) never
            # leave the in-order PE queue head blocked — while ACT works
            # on pair p, the PE retires pv(p-2) and scores(p+1/p+2).
            for p in range(len(pairs)):
                emit_scores(p)
                if p == 4:
                    emit_vt(1)
                if p >= 2 and stage != "nopv":
                    emit_pv(p - 2)
            if pairs and stage != "nopv":
                emit_pv(len(pairs) - 2)
                emit_pv(len(pairs) - 1)
            if stage == "nopv":
                # consume es tiles so the pool rotation still closes
                cons = bigp.tile([P, 4 * QB], bf16, name="cons", tag="cons")
                for p in list(es_t):
                    nc.vector.tensor_copy(cons[:], es_t.pop(p)[:])
                ot_t.clear()
            for i in (range(4) if pairs and stage != "nopv" else []):
                emit_out(i, ot_t.pop(i))
                if i == 2:
                    # qb0-2 rows ship (via the idle SWDGE ring) while
                    # qb3's tail is still in flight
                    nc.gpsimd.dma_start(yv[:, 0:6, :], ys[:, 0:6, :])
            if pairs and stage != "nopv":
                nc.sync.dma_start(yv[:, 6:NCB, :], ys[:, 6:NCB, :])
            elif stage == "nopv":
                nc.sync.dma_start(yv[:, 6:NCB, :],
                                  cons[:, 0:2 * H].rearrange(
                                      "p (c h) -> p c h", c=2))

    nc.compile()
    return nc


def _make_masks():
    i = np.arange(P)[:, None]
    j = np.arange(QB)[None, :]
    ma = (i <= j).astype(BF16)
    mb = (i + P <= j).astype(BF16)
    return ma, mb


def make_in_maps(x, Wq, Wk, Wv):
    """Per-core input dicts. Core 2*b + t owns query chunks {t, t+2, t+4, t+6}.

    kv layout after the rank-ordered AllGather is global-fixed:
    kvT[:, 0, :] = even-chunk rows (core 2b's shard, global chunks
    {0,2,4,6} in own-local order), kvT[:, 1, :] = odd chunks. For
    query-block i (global chunk g = 2i+t) the program masks the last even
    unit pair and the last odd unit pair:
      t=0: even pair i is the diagonal (Ma/Mb), odd pair i is acausal (0);
      t=1: even pair i is fully valid (1), odd pair i is the diagonal.
    """
    x = np.asarray(x, np.float32)
    w = np.concatenate([np.asarray(Wq, np.float32),
                        np.asarray(Wk, np.float32),
                        np.asarray(Wv, np.float32)], axis=1)  # [C, 3H]
    w = np.ascontiguousarray(
        w.reshape(NCB, P, 3 * H).transpose(1, 0, 2).reshape(P, NCB * 3 * H)
    ).astype(BF16)
    iden = np.eye(P, dtype=BF16)
    ma, mb = _make_masks()
    ones = np.ones((P, QB), BF16)
    zeros = np.zeros((P, QB), BF16)
    xc = x.reshape(B, 8, QB, C)
    in_maps = []
    for core in range(NCORES):
        b, t = divmod(core, 2)
        own = [2 * k + t for k in range(4)]
        xqT = np.ascontiguousarray(
            xc[b, own].reshape(TQ, C).T).astype(BF16)
        if t == 0:
            mk = np.concatenate([iden, ma, mb, zeros, zeros], axis=1)
        else:
            mk = np.concatenate([iden, ones, ones, ma, mb], axis=1)
        in_maps.append({
            "xqT": xqT, "wqkv": w, "consts": mk,
        })
    return in_maps


def assemble(results):
    y = np.empty((B, T, H), np.float32)
    for core in range(NCORES):
        b, t = divmod(core, 2)
        yc = np.asarray(results[core]["y"]).astype(np.float32)
        for i in range(4):
            g = 2 * i + t
            y[b, g * QB:(g + 1) * QB, :] = yc[i * QB:(i + 1) * QB, :]
    return y


def kernel(x, Wq, Wk, Wv):
    from concourse.bass_utils import run_bass_kernel_spmd
    if "nc" not in _CACHE:
        _CACHE["nc"] = _build_program()
    nc = _CACHE["nc"]
    in_maps = make_in_maps(x, Wq, Wk, Wv)
    res = run_bass_kernel_spmd(nc, in_maps, list(range(NCORES)))
    return assemble(res.results)


# revision 20
# speedup vs baseline: 1.1068x; 1.1068x over previous
"""Trainium2 Bass kernel for a single-head causal attention block.

Reference computation (B=4, T=2048, C=1024, H=64):
    q = x @ Wq; k = x @ Wk; v = x @ Wv          # [B,T,H]
    scores = (q @ k^T) * C**-0.5                # causal masked
    out = softmax(scores) @ v                   # [B,T,H]

Sharding: 2 cores per batch (8 cores, B=4). Core (b, t) owns the 4
interleaved 256-row query chunks {t, t+2, t+4, t+6} of batch b, which
balances causal work exactly across the pair. One uniform SPMD program;
all per-core differences are input data (row arrangement + 0/1 masks).

On-chip pipeline (bf16 on-chip, fp32 PSUM accumulate; rel err ~5e-3):
  x is pre-transposed on the host, so x^T streams in as plain contiguous
  DMAs, split across the SP and ACT HWDGE queues -> K^T/V^T + Q^T
  projections off one packed [Wq|Wk|Wv] stationary -> per-512-row-half
  exchange of the packed [K^T|V^T] with the pair partner: AllGather
  through a flat DRAM buffer (fake_cc: equivalent-volume local DMAs for
  the benchmark loop; half 0 runs on the ACT queue, half 1 on SP) ->
  rank-ordered [even-chunk | odd-chunk] kv layout, uniform across
  cores -> V^T unpacked to V-natural via PE transposes -> S^T = K.Q^T
  tiles [s=128, q=256] -> one exp per pair group on ScalarE (1/32 scale
  folded in; logits tiny, no max-subtraction) -> causal masks via DVE
  multiplies -> P^T.[V|1|0] matmul accumulates out^T and the softmax
  denominator together -> PE transpose + reciprocal -> bf16 y rows
  staged and stored via the idle Pool/SWDGE queue.
"""

import numpy as np
import ml_dtypes

B, T, C, H = 4, 2048, 1024, 64
NCORES = 8
P = 128          # partitions
NCB = C // P     # 8 channel blocks
QB = 256         # query block width
TQ = T // 2      # own query rows per core
SCALE = float(C) ** -0.5
BF16 = ml_dtypes.bfloat16

# flat exchange payload per 512-row half: [K^T | V^T] = 128x512 bf16
EX_ELE = 128 * 512

_CACHE = {}


def _build_program(loop_n=1, fake_cc=False, stage="full"):
    # fake_cc: replace the AllGather with equivalent-volume local DMAs —
    # numerically wrong (peer half duplicated) but timing-equivalent; used
    # only by the For_i benchmark loop, where real collectives desync.
    import contextlib
    import concourse.bacc as bacc
    import concourse.mybir as mybir
    from concourse import tile

    f32 = mybir.dt.float32
    bf16 = mybir.dt.bfloat16
    EXP = mybir.ActivationFunctionType.Exp

    nc = bacc.Bacc("TRN2", target_bir_lowering=False, debug=False,
                   num_devices=NCORES)

    xqT_d = nc.dram_tensor("xqT", [C, TQ], bf16, kind="ExternalInput").ap()
    w_d = nc.dram_tensor("wqkv", [P, NCB * 3 * H], bf16,
                         kind="ExternalInput").ap()
    # consts: [iden 128 | masks 4x256] packed in one tensor/DMA
    const_d = nc.dram_tensor("consts", [P, P + 4 * QB], bf16,
                             kind="ExternalInput").ap()
    y_d = nc.dram_tensor("y", [TQ, H], bf16, kind="ExternalOutput").ap()

    with tile.TileContext(nc) as tc:
        with (
            tc.tile_pool(name="const", bufs=1) as constp,
            tc.tile_pool(name="big", bufs=1) as bigp,
            tc.tile_pool(name="exps", bufs=4) as expp,
            tc.tile_pool(name="small", bufs=4) as smallp,
            tc.tile_pool(name="pt", bufs=2, space="PSUM") as psum_t,
            tc.tile_pool(name="psc", bufs=2, space="PSUM") as psum_sc,
            tc.tile_pool(name="po", bufs=2, space="PSUM") as psum_o,
            tc.tile_pool(name="dram", bufs=1, space="DRAM") as dramp,
        ):
          with (tc.For_i(0, loop_n, 1,
                         hint_engines=(mybir.EngineType.PE,
                                       mybir.EngineType.SP,
                                       mybir.EngineType.Activation,
                                       mybir.EngineType.DVE,
                                       mybir.EngineType.Pool))
                if loop_n > 1 else contextlib.nullcontext()):
            w_s = constp.tile([P, NCB, 3 * H], bf16)
            nc.gpsimd.dma_start(
                w_s[:], w_d.rearrange("p (c o) -> p c o", c=NCB))
            consts = constp.tile([P, P + 4 * QB], bf16, name="consts_s",
                                 tag="consts_s")
            nc.gpsimd.dma_start(consts[:], const_d)
            iden = consts[:, 0:P]
            mask_s = {m: consts[:, P + j * QB:P + (j + 1) * QB]
                      for j, m in enumerate("abcd")}
            zbias = constp.tile([P, 1], f32)
            nc.vector.memset(zbias[:], 0.0)
            # warm the ACT exp table-set early (one-time ~2.7us DMA load
            # otherwise lands on the attention critical path)
            expwarm = constp.tile([P, 1], f32)
            nc.scalar.activation(expwarm[:], zbias[:], EXP, bias=zbias[:])

            # ---- x^T: plain loads of the host-transposed x ----
            # [c_part, cb, own_row]; each (cb, half) is one [128, 512]
            # contiguous-row DMA; cb 0-3 ride the SP queue, 4-7 the ACT
            # queue, so the two HWDGE rings fill in parallel.
            xT = bigp.tile([P, NCB, TQ], bf16, name="xT", tag="xT")

            def load_x_half(h):
                for cb in range(NCB):
                    eng = nc.sync if cb < 4 else nc.scalar
                    eng.dma_start(
                        xT[:, cb, h * 512:(h + 1) * 512],
                        xqT_d[cb * P:(cb + 1) * P, h * 512:(h + 1) * 512])

            if stage != "attn":
                load_x_half(0)
                load_x_half(1)

            # ---- projections + per-half [K^T|V^T] exchange ----
            kvT = bigp.tile([P, 2, TQ], bf16, name="kvT", tag="kvT")
            qT = bigp.tile([H, TQ], bf16, name="qT", tag="qT")
            kv_own = bigp.tile([P, TQ], bf16, name="kv_own", tag="kv_own")
            vp = {par: bigp.tile([P, NCB, H + 2], bf16, name=f"vp{par}",
                                 tag=f"vp{par}") for par in range(2)}
            for par in range(2):
                nc.vector.memset(vp[par][:, :, H:H + 1], 1.0)
                nc.vector.memset(vp[par][:, :, H + 1:H + 2], 0.0)

            in_cc = [dramp.tile([1, EX_ELE], bf16, name=f"incc{h}",
                                tag=f"incc{h}") for h in range(2)]
            out_cc = [dramp.tile([2, EX_ELE], bf16, name=f"outcc{h}",
                                 tag=f"outcc{h}") for h in range(2)]

            if stage == "attn":
                nc.vector.memset(kvT[:, :, :], 0.01)
                nc.vector.memset(qT[:, :], 0.01)
                nc.vector.memset(kv_own[:, :], 0.01)
            for h in range(([] if stage == "attn" else [0, 1]) and 2 or 0) if False else range(0 if stage == "attn" else 2):
                cols = slice(h * 512, (h + 1) * 512)
                # exchange half 0 on the ACT ring (idle pre-attention),
                # half 1 on SP (free once x is in)
                ex_eng = nc.scalar if h == 0 else nc.sync
                # K^T/V^T packed projection: psum [k|v, 512] f32
                pkv = psum_t.tile([P, 512], f32, tag="pt")
                for cb in range(NCB):
                    nc.tensor.matmul(
                        pkv[:], w_s[:, cb, H:3 * H], xT[:, cb, cols],
                        start=(cb == 0), stop=(cb == NCB - 1))
                nc.vector.tensor_copy(kv_own[:, cols], pkv[:])
                # Q^T projection
                pq = psum_t.tile([H, 512], f32, tag="pt", name=f"pq{h}")
                for cb in range(NCB):
                    nc.tensor.matmul(
                        pq[:], w_s[:, cb, 0:H], xT[:, cb, cols],
                        start=(cb == 0), stop=(cb == NCB - 1))
                nc.scalar.copy(qT[:, cols], pq[:])

                # ship this half's [K^T | V^T] to the pair partner
                in_v = in_cc[h][0:1, :].rearrange("a (p q) -> (a p) q", p=P)
                if fake_cc:
                    # equivalent-volume local writes standing in for the
                    # AllGather's two output-shard writes
                    for par in range(2):
                        ex_eng.dma_start(
                            out_cc[h][par:par + 1, :].rearrange(
                                "a (p q) -> (a p) q", p=P),
                            kv_own[:, cols])
                    ex_eng.dma_start(in_v, kv_own[:, cols])
                else:
                    ex_eng.dma_start(in_v, kv_own[:, cols])
                    nc.gpsimd.collective_compute(
                        "AllGather",
                        mybir.AluOpType.bypass,
                        replica_groups=[[2 * b, 2 * b + 1]
                                        for b in range(NCORES // 2)],
                        ins=[in_cc[h].opt()],
                        outs=[out_cc[h].opt()],
                    )
                # rank-ordered kv: [:, 0, :] even-chunk [K^T|V^T], [:, 1, :]
                # odd — both shards land in one DMA
                ex_eng.dma_start(
                    kvT[:, 0:2, cols],
                    out_cc[h][0:2, :].rearrange("r (p q) -> p r q", p=P))

            # V^T -> V natural into vp (8 PE transposes of [64,128] per
            # half). h=0 is emitted here; h=1 is deferred into the pair
            # stream (after pair (2,0)'s scores) so its exchange-load
            # dependency does not stall earlier attention work in the
            # in-order PE queue.
            def emit_vt(h):
                for par in range(2):
                    for u in range(4):
                        pv = psum_t.tile([P, H], bf16, tag="pt",
                                         name=f"pv{h}{par}{u}")
                        nc.tensor.transpose(
                            pv[:], kvT[H:2 * H, par, h * 512 + u * P:
                                       h * 512 + (u + 1) * P],
                            iden[H:2 * H, H:2 * H])
                        nc.vector.tensor_copy(
                            vp[par][:, 4 * h + u, 0:H], pv[:])

            emit_vt(0)

            if stage == "front":
                ysf = bigp.tile([P, 16], bf16, name="ysf", tag="ysf")
                nc.vector.tensor_copy(ysf[:], kvT[:, 0, 0:16])
                nc.sync.dma_start(y_d[0:P, 0:16], ysf[:])
            # ---- attention: software-pipelined across (qb, group) ----
            # The PE runs in program order, so scores for stage p+1 are
            # emitted BEFORE the PV matmuls of stage p; the exp/mask chain
            # of stage p then overlaps the next scores group instead of
            # stalling the PE.
            pairs = []
            for i in range(4):
                # unit list: even-chunk units 0..2i+1 then odd-chunk units
                seq = ([(0, u) for u in range(2 * i + 2)]
                       + [(1, u) for u in range(2 * i + 2)])
                for g in range(i + 1):
                    pairs.append((i, g, i + 1, seq))

            po_t = {}
            es_t = {}

            def emit_scores(p):
                i, g, G, seq = pairs[p]
                ps = psum_sc.tile([P, 4 * QB], f32, tag="ps", name=f"ps{p}")
                for k in range(4):
                    par, u = seq[4 * g + k]
                    nc.tensor.matmul(
                        ps[:, k * QB:(k + 1) * QB],
                        kvT[0:H, par, u * P:(u + 1) * P],
                        qT[0:H, i * QB:(i + 1) * QB],
                        start=True, stop=True)
                es = expp.tile([P, 4 * QB], bf16, tag="es", name=f"es{p}")
                if stage == "noexp":
                    nc.vector.tensor_copy(es[:], ps[:])
                elif stage == "smallexp":
                    for eh in range(2):
                        sl = slice(eh * 2 * QB, (eh + 1) * 2 * QB)
                        nc.scalar.activation(es[:, sl], ps[:, sl], EXP,
                                             bias=zbias[:], scale=SCALE)
                else:
                    nc.scalar.activation(es[:], ps[:], EXP,
                                         bias=zbias[:], scale=SCALE)
                if stage != "nomask":
                    for k in range(4):
                        pos = 4 * g + k
                        m = {2 * i: "a", 2 * i + 1: "b",
                             4 * i + 2: "c", 4 * i + 3: "d"}.get(pos)
                        if m is not None:
                            sl = es[:, k * QB:(k + 1) * QB]
                            nc.vector.tensor_mul(sl, sl, mask_s[m])
                es_t[p] = es

            def emit_pv(p):
                i, g, G, seq = pairs[p]
                if g == 0:
                    po_t[i] = psum_o.tile([H + 2, QB], f32, tag="po",
                                          name=f"po{i}")
                es = es_t.pop(p)
                for k in range(4):
                    par, u = seq[4 * g + k]
                    nc.tensor.matmul(
                        po_t[i][:], vp[par][:, u, 0:H + 2],
                        es[:, k * QB:(k + 1) * QB],
                        start=(g == 0 and k == 0),
                        stop=(g == G - 1 and k == 3))
                if g == G - 1:
                    # release po promptly (DVE copy), but DEFER the PE
                    # transpose + output math so it does not block the
                    # next scores group in the PE's in-order stream
                    po = po_t.pop(i)
                    ot = smallp.tile([H + 2, QB], bf16, tag="ot",
                                     name=f"ot{i}")
                    nc.vector.tensor_copy(ot[:], po[:])
                    ot_t[i] = ot

            ot_t = {}
            ys = bigp.tile([P, NCB, H], bf16, name="ys", tag="ys")

            def emit_out(i, ot):
                for h2 in range(2):
                    pt2 = psum_t.tile([P, H + 2], bf16, tag="pt",
                                      name=f"pt2_{i}_{h2}")
                    nc.tensor.transpose(
                        pt2[:], ot[0:H + 2, h2 * P:(h2 + 1) * P],
                        iden[0:H + 2, 0:H + 2])
                    rc = smallp.tile([P, 1], f32, tag="rc",
                                     name=f"rc{i}{h2}")
                    nc.vector.reciprocal(rc[:], pt2[:, H:H + 1])
                    nc.vector.tensor_scalar_mul(
                        ys[:, 2 * i + h2, :], pt2[:, 0:H], rc[:])

            yv = y_d.rearrange("(c p) h -> p c h", p=P)
            if stage == "front":
                pairs = []
            # depth-3 software pipeline: the PE stream runs scores three
            # pairs ahead of the PV consumers. By the time the in-order PE
            # queue reaches pv(p), exp(p)+masks(p) finished during the
            # following scores groups, so pv never blocks the queue head;
            # the only PE stall is the ps-buffer rotation (= ACT pacing).
            for p in range(len(pairs)):
                emit_scores(p)
                if p == 5:
                    emit_vt(1)
                if p >= 3 and stage != "nopv":
                    emit_pv(p - 3)
            if pairs and stage != "nopv":
                for p in range(len(pairs) - 3, len(pairs)):
                    emit_pv(p)
            if stage == "nopv":
                # consume es tiles so the pool rotation still closes
                cons = bigp.tile([P, 4 * QB], bf16, name="cons", tag="cons")
                for p in list(es_t):
                    nc.vector.tensor_copy(cons[:], es_t.pop(p)[:])
                ot_t.clear()
            for i in (range(4) if pairs and stage != "nopv" else []):
                emit_out(i, ot_t.pop(i))
                if i == 2:
                    # qb0-2 rows ship (via the idle SWDGE ring) while
                    # qb3's tail is still in flight
                    nc.gpsimd.dma_start(yv[:, 0:6, :], ys[:, 0:6, :])
            if pairs and stage != "nopv":
                nc.sync.dma_start(yv[:, 6:NCB, :], ys[:, 6:NCB, :])
            elif stage == "nopv":
                nc.sync.dma_start(yv[:, 6:NCB, :],
                                  cons[:, 0:2 * H].rearrange(
                                      "p (c h) -> p c h", c=2))

    nc.compile()
    return nc


def _make_masks():
    i = np.arange(P)[:, None]
    j = np.arange(QB)[None, :]
    ma = (i <= j).astype(BF16)
    mb = (i + P <= j).astype(BF16)
    return ma, mb


def make_in_maps(x, Wq, Wk, Wv):
    """Per-core input dicts. Core 2*b + t owns query chunks {t, t+2, t+4, t+6}.

    kv layout after the rank-ordered AllGather is global-fixed:
    kvT[:, 0, :] = even-chunk rows (core 2b's shard, global chunks
    {0,2,4,6} in own-local order), kvT[:, 1, :] = odd chunks. For
    query-block i (global chunk g = 2i+t) the program masks the last even
    unit pair and the last odd unit pair:
      t=0: even pair i is the diagonal (Ma/Mb), odd pair i is acausal (0);
      t=1: even pair i is fully valid (1), odd pair i is the diagonal.
    """
    x = np.asarray(x, np.float32)
    w = np.concatenate([np.asarray(Wq, np.float32),
                        np.asarray(Wk, np.float32),
                        np.asarray(Wv, np.float32)], axis=1)  # [C, 3H]
    w = np.ascontiguousarray(
        w.reshape(NCB, P, 3 * H).transpose(1, 0, 2).reshape(P, NCB * 3 * H)
    ).astype(BF16)
    iden = np.eye(P, dtype=BF16)
    ma, mb = _make_masks()
    ones = np.ones((P, QB), BF16)
    zeros = np.zeros((P, QB), BF16)
    xc = x.reshape(B, 8, QB, C)
    in_maps = []
    for core in range(NCORES):
        b, t = divmod(core, 2)
        own = [2 * k + t for k in range(4)]
        xqT = np.ascontiguousarray(
            xc[b, own].reshape(TQ, C).T).astype(BF16)
        if t == 0:
            mk = np.concatenate([iden, ma, mb, zeros, zeros], axis=1)
        else:
            mk = np.concatenate([iden, ones, ones, ma, mb], axis=1)
        in_maps.append({
            "xqT": xqT, "wqkv": w, "consts": mk,
        })
    return in_maps


def assemble(results):
    y = np.empty((B, T, H), np.float32)
    for core in range(NCORES):
        b, t = divmod(core, 2)
        yc = np.asarray(results[core]["y"]).astype(np.float32)
        for i in range(4):
            g = 2 * i + t
            y[b, g * QB:(g + 1) * QB, :] = yc[i * QB:(i + 1) * QB, :]
    return y


def kernel(x, Wq, Wk, Wv):
    from concourse.bass_utils import run_bass_kernel_spmd
    if "nc" not in _CACHE:
        _CACHE["nc"] = _build_program()
    nc = _CACHE["nc"]
    in_maps = make_in_maps(x, Wq, Wk, Wv)
    res = run_bass_kernel_spmd(nc, in_maps, list(range(NCORES)))
    return assemble(res.results)
